# revision 13
# baseline (speedup 1.0000x reference)
"""Trainium2 Bass kernel for NNBlendFM: 3-layer tanh MLP embedder + 64-head
rank-16 factorization machine, data-parallel over batch across 8 NeuronCores.

Math (per batch row b, head h):
    h = tanh(tanh(tanh(x W1 + b1) W2 + b2) W3 + b3)          # [B, 2048]
    lin[b,h]  = h . fm_w[h]
    vx[b,h,r] = h . fm_V[h,r]
    diag[b,h] = (h*h) . (sum_r fm_V[h,r]^2)
    out[h,b]  = fm_w0[h] + lin + 0.5*(sum_r vx^2 - diag)

All matmul operands are bf16 (fp32 PSUM accumulation).  fp16 was measured
~20% SLOWER per matmul row on this PE despite the cost model's 1.0
cycles/row claim, so bf16 it is; its 4.1e-3 rel err clears the 2e-2 gate.

Schedule notes (from trace analysis):
  - The PE stream is ~737.8K moving rows ~= 307us at 2.4 GHz; everything else
    is head/tail.  DMA descriptors drain in ~global issue order, so the head
    set (b1, x, W1) is issued first, and the 17MB W2/W3/VT stream follows on
    the SAME sync queue so it cannot starve the layer-1 inputs.
  - One 4KB-slot weight pool (32 bufs): alloc order warm, W1, W2, W3, VT
    (k-pair packed), FWSQ.  The ring makes W3's last five tiles alias the
    warm-up/W1 buffers (free ~35us) instead of W2 (free only at L2 end),
    and VT alias W2 (first read ~266us) — no more W3-wait stalls in L3.
  - PE warm-up matmuls on a vector-memset tile ramp HAM (1.2 -> 2.4 GHz)
    during the DMA head.
"""

import numpy as np
import ml_dtypes

import concourse.tile as tile
from concourse import bacc, mybir
from concourse import bass_utils

BF16 = mybir.dt.bfloat16
F32 = mybir.dt.float32
AF = mybir.ActivationFunctionType
ALU = mybir.AluOpType

P = 128
IN, HID, HEADS, RANK = 512, 2048, 64, 16
B = 8192
NCORES = 8
BC = B // NCORES            # 1024 batch rows per core
KT1 = IN // P               # 4  k-tiles, layer 1
KT = HID // P               # 16 k-tiles, layers 2/3 + FM
JT = HID // P               # 16 output-feature tiles per layer
NB = 512                    # matmul moving free-dim (one PSUM bank)
NBC = BC // NB              # 2 batch column chunks
BT = BC // P                # 8 batch tiles in FM stage
HR = HEADS * RANK           # 1024 vx columns

_CACHE = {}


def _build_module():
    nc = bacc.Bacc(
        "TRN2", target_bir_lowering=False, debug=False, num_devices=NCORES
    )
    dt = nc.dram_tensor
    xT = dt("xT", [IN, BC], BF16, kind="ExternalInput").ap()
    W1 = dt("W1", [IN, HID], BF16, kind="ExternalInput").ap()
    W2 = dt("W2", [HID, HID], BF16, kind="ExternalInput").ap()
    W3 = dt("W3", [HID, HID], BF16, kind="ExternalInput").ap()
    B1 = dt("B1", [P, JT], F32, kind="ExternalInput").ap()
    B2 = dt("B2", [P, JT], F32, kind="ExternalInput").ap()
    B3 = dt("B3", [P, JT], F32, kind="ExternalInput").ap()
    # V^T packed as k-pairs: VTP[p, kp*2*HR + i*HR + hr] = VT[(2kp+i)*128+p, hr]
    VTP = dt("VTP", [P, (KT // 2) * 2 * HR], BF16, kind="ExternalInput").ap()
    # fm_w^T and 0.5*sum_r V^2 packed side by side in one [128, 2*KT*HEADS]
    FWSQ = dt("FWSQ", [P, 2 * KT * HEADS], BF16, kind="ExternalInput").ap()
    W0C = dt("W0C", [P, HEADS], BF16, kind="ExternalInput").ap()
    OUT = dt("out", [BC, HEADS], F32, kind="ExternalOutput").ap()

    with tile.TileContext(nc) as tc:
        with (
            tc.tile_pool(name="wpool", bufs=32) as wpool,   # 4KB slots
            tc.tile_pool(name="hpool", bufs=32) as hpool,   # 2KB slots
            tc.tile_pool(name="cpool", bufs=1) as cpool,
            tc.tile_pool(name="pp", bufs=8, space="PSUM") as pp,
            tc.tile_pool(name="epool", bufs=2) as epool,
            tc.tile_pool(name="spool", bufs=8) as spool,
            tc.tile_pool(name="opool", bufs=4) as opool,
        ):
            # --- wpool alloc order defines the alias ring (32 bufs):
            #   0: warm, 1-4: W1, 5-20: W2, 21-36: W3 (33-36 alias warm+W1),
            #   37-44: VT pairs (alias W2_0..7), 45: FWSQ (aliases W2_8).
            warm = wpool.tile([P, HID], BF16, tag="w", name="warm")
            nc.vector.memset(warm[:, 0:NB], 0.0)

            # DMA descriptors drain in roughly global issue order, so the
            # program-order of dma_starts IS the fabric priority.  The whole
            # head set goes on sync in exact need-order (b1, x/W1 pairs),
            # with x2/x3 on scalar in parallel to save issue latency.  The
            # heavy W2/W3/VT stream follows ON THE SAME sync queue so it
            # cannot get ahead of the layer-1 inputs.
            b1t = cpool.tile([P, JT], F32, tag="b1")
            xt = [
                hpool.tile([P, BC], BF16, tag="h", name=f"xt{k}")
                for k in range(KT1)
            ]
            # W1's kt0 row-block is column-split into two tiles so the very
            # first matmul (jt0, kt0) only waits for a 256KB half, making the
            # kernel start x0-bound.  jt 0-7 read w1a0, jt 8-15 read w1b0.
            w1a0 = wpool.tile([P, HID // 2], BF16, tag="w", name="w1a0")
            w1b0 = wpool.tile([P, HID // 2], BF16, tag="w", name="w1b0")
            w1t = [None] + [
                wpool.tile([P, HID], BF16, tag="w", name=f"w1_{k}")
                for k in range(1, KT1)
            ]
            # Three queues drain concurrently in need order: x0 leads
            # gpsimd (earliest-ready queue), x1-x3 on sync, W1 on scalar.
            nc.gpsimd.dma_start(xt[0][:], xT[0:P, :])
            for k in range(1, KT1):
                nc.sync.dma_start(xt[k][:], xT[k * P : (k + 1) * P, :])
            nc.scalar.dma_start(w1a0[:], W1[0:P, 0 : HID // 2])
            for k in range(1, KT1):
                nc.scalar.dma_start(w1t[k][:], W1[k * P : (k + 1) * P, :])
            nc.scalar.dma_start(w1b0[:], W1[0:P, HID // 2 : HID])

            nc.gpsimd.dma_start(b1t[:], B1)

            def l1_lhsT(kt, jt):
                if kt == 0:
                    half, j = (w1a0, jt) if jt < 8 else (w1b0, jt - 8)
                    return half[:, j * P : (j + 1) * P]
                return w1t[kt][:, jt * P : (jt + 1) * P]

            # PE warm-up: ramps the clock while the head DMAs fly.
            wu = pp.tile([P, NB], F32, tag="ps", name="warm")
            for _ in range(4):
                nc.tensor.matmul(
                    wu[:], warm[:, 0:P], warm[:, 0:NB], start=True, stop=True
                )

            # Non-critical small constants on gpsimd (tiny transfers).
            b2t = cpool.tile([P, JT], F32, tag="b2")
            nc.gpsimd.dma_start(b2t[:], B2)
            b3t = cpool.tile([P, JT], F32, tag="b3")
            nc.gpsimd.dma_start(b3t[:], B3)
            # -w0/128 replicated; contracted against a ones column block so
            # the diag PSUM group finishes as (0.5*diag - w0).
            w0c = cpool.tile([P, HEADS], BF16, tag="w0c")
            nc.gpsimd.dma_start(w0c[:], W0C)
            onest = cpool.tile([P, P], BF16, tag="ones")
            nc.vector.memset(onest[:], 1.0)

            # Heavy weights in strict priority order on sync's ring.
            def load_w(dram, ktiles, name):
                ts = []
                for k in range(ktiles):
                    w_k = wpool.tile([P, HID], BF16, tag="w", name=f"{name}_{k}")
                    nc.sync.dma_start(w_k[:], dram[k * P : (k + 1) * P, :])
                    ts.append(w_k)
                return ts

            w2t = load_w(W2, KT, "w2")
            w3t = load_w(W3, KT, "w3")
            vtp = []
            for kp in range(KT // 2):
                v_k = wpool.tile([P, 2 * HR], BF16, tag="w", name=f"vt{kp}")
                nc.sync.dma_start(
                    v_k[:], VTP[:, kp * 2 * HR : (kp + 1) * 2 * HR]
                )
                vtp.append(v_k)

            def vtt(kt):                       # [128, HR] view of V^T k-tile
                return vtp[kt // 2][:, (kt % 2) * HR : (kt % 2 + 1) * HR]

            fwsq = wpool.tile([P, 2 * KT * HEADS], BF16, tag="w", name="fwsq")
            nc.sync.dma_start(fwsq[:], FWSQ)
            fwt = fwsq[:, 0 : KT * HEADS]
            sqt = fwsq[:, KT * HEADS : 2 * KT * HEADS]

            # --- embedder layers ------------------------------------------
            def layer(h_prev, lhsT_fn, bias_t, ktiles, name, rot):
                h_out = []
                for jt in range(JT):
                    ps = []
                    for c in range(NBC):
                        ps_c = pp.tile([P, NB], F32, tag="ps", name=f"{name}ps{jt}_{c}")
                        ps.append(ps_c)
                    # Rotate the accumulation start by jt (mod rot) so every
                    # jt can begin with an already-arrived k-tile while later
                    # tiles stream in.
                    kts = [(kt + jt) % rot for kt in range(rot)] + list(
                        range(rot, ktiles)
                    )
                    for i, kt in enumerate(kts):
                        lhsT = lhsT_fn(kt, jt)
                        for c in range(NBC):
                            nc.tensor.matmul(
                                ps[c][:],
                                lhsT,
                                h_prev[kt][:, c * NB : (c + 1) * NB],
                                start=(i == 0),
                                stop=(i == ktiles - 1),
                            )
                    ht = hpool.tile([P, BC], BF16, tag="h", name=f"{name}h{jt}")
                    for c in range(NBC):
                        nc.scalar.activation(
                            ht[:, c * NB : (c + 1) * NB],
                            ps[c][:],
                            AF.Tanh,
                            bias=bias_t[:, jt : jt + 1],
                        )
                    h_out.append(ht)
                return h_out

            def w_lhsT(tiles):
                return lambda kt, jt: tiles[kt][:, jt * P : (jt + 1) * P]

            h1 = layer(xt, l1_lhsT, b1t, KT1, "l1", rot=2)
            h2 = layer(h1, w_lhsT(w2t), b2t, KT, "l2", rot=KT)
            h3 = layer(h2, w_lhsT(w3t), b3t, KT, "l3", rot=KT)

            # --- h3 squared (stationary operand for the diag matmuls) -----
            h3sq = []
            for k in range(KT):
                sq_k = hpool.tile([P, BC], BF16, tag="h", name=f"h3sq{k}")
                nc.vector.tensor_mul(sq_k[:], h3[k][:], h3[k][:])
                h3sq.append(sq_k)

            # --- FM stage: per 128-row batch tile -------------------------
            def fm_phase_a(bt):
                """vx = h V^T (1024 cols) and lin = h fm_w^T (64 cols)."""
                vx0 = pp.tile([P, NB], F32, tag="ps", name=f"vx0_{bt}")
                vx1 = pp.tile([P, NB], F32, tag="ps", name=f"vx1_{bt}")
                lw = pp.tile([P, NB], F32, tag="ps", name=f"lw_{bt}")
                bsl = slice(bt * P, (bt + 1) * P)
                for kt in range(KT):
                    lhsT = h3[kt][:, bsl]
                    vt_k = vtt(kt)
                    nc.tensor.matmul(
                        vx0[:], lhsT, vt_k[:, 0:NB],
                        start=(kt == 0), stop=(kt == KT - 1),
                    )
                    nc.tensor.matmul(
                        vx1[:], lhsT, vt_k[:, NB:HR],
                        start=(kt == 0), stop=(kt == KT - 1),
                    )
                    nc.tensor.matmul(
                        lw[:, 0:HEADS], lhsT,
                        fwt[:, kt * HEADS : (kt + 1) * HEADS],
                        start=(kt == 0), stop=(kt == KT - 1),
                    )
                return vx0, vx1, lw

            def fm_phase_b(bt):
                """diag = (h*h) . (0.5 * sum_r V^2), already scaled by 0.5."""
                dg = pp.tile([P, NB], F32, tag="ps", name=f"dg_{bt}")
                bsl = slice(bt * P, (bt + 1) * P)
                for kt in range(KT):
                    nc.tensor.matmul(
                        dg[:, 0:HEADS],
                        h3sq[kt][:, bsl],
                        sqt[:, kt * HEADS : (kt + 1) * HEADS],
                        start=(kt == 0), stop=False,
                    )
                nc.tensor.matmul(
                    dg[:, 0:HEADS], onest[:], w0c[:], start=False, stop=True,
                )
                return dg

            def fm_square_reduce(bt, vx0, vx1):
                """Emitted right after phase A: overlaps later bt's matmuls.
                Each 512-wide half squares then reduces independently so the
                two chains pipeline across ACT and DVE."""
                vx2 = epool.tile([P, HR], BF16, tag="e", name=f"vx2_{bt}")
                sumv = spool.tile([P, HEADS], F32, tag="s", name=f"sumv_{bt}")
                for c, vxh in ((0, vx0), (1, vx1)):
                    nc.scalar.activation(vx2[:, c * NB : (c + 1) * NB], vxh[:], AF.Square)
                    nc.vector.reduce_sum(
                        sumv[:, c * (HEADS // 2) : (c + 1) * (HEADS // 2)],
                        vx2[:, c * NB : (c + 1) * NB].rearrange(
                            "p (h r) -> p h r", r=RANK
                        ),
                        axis=mybir.AxisListType.X,
                    )
                return sumv

            def fm_combine(bt, sumv, lw, dg):
                # q = 0.5*sumv - diag_half
                q = spool.tile([P, HEADS], F32, tag="s", name=f"q_{bt}")
                nc.vector.scalar_tensor_tensor(
                    q[:], sumv[:], 0.5, dg[:, 0:HEADS],
                    op0=ALU.mult, op1=ALU.subtract,
                )
                ot = opool.tile([P, HEADS], F32, tag="o", name=f"ot_{bt}")
                nc.vector.tensor_add(ot[:], q[:], lw[:, 0:HEADS])
                nc.sync.dma_start(OUT[bt * P : (bt + 1) * P, :], ot[:])

            # Stagger: A(0), A(1), B(0), C(0), A(2), B(1), C(1), ...
            pend = []  # (bt, sumv, lw)
            for bt in range(BT):
                vx0, vx1, lw = fm_phase_a(bt)
                sumv = fm_square_reduce(bt, vx0, vx1)
                pend.append((bt, sumv, lw))
                if len(pend) == 2:
                    obt, osumv, olw = pend.pop(0)
                    dg = fm_phase_b(obt)
                    fm_combine(obt, osumv, olw, dg)
            while pend:
                obt, osumv, olw = pend.pop(0)
                dg = fm_phase_b(obt)
                fm_combine(obt, osumv, olw, dg)

    nc.compile()
    return nc


def _get_nc():
    if "nc" not in _CACHE:
        _CACHE["nc"] = _build_module()
    return _CACHE["nc"]


def _prep_host(x, W1, b1, W2, b2, W3, b3, fm_w0, fm_w, fm_V):
    """Host-side layout prep: bf16 casts, transposes, per-head V reductions."""
    bf = ml_dtypes.bfloat16
    f32 = np.float32

    # V^T: [2048, heads*rank], col hr = h*RANK + r; then k-pair packed
    VT = fm_V.reshape(HEADS * RANK, HID).T.astype(bf)          # [2048, 1024]
    VTP = np.empty((P, (KT // 2) * 2 * HR), dtype=bf)
    for kp in range(KT // 2):
        for i in range(2):
            VTP[:, kp * 2 * HR + i * HR : kp * 2 * HR + (i + 1) * HR] = VT[
                (2 * kp + i) * P : (2 * kp + i + 1) * P, :
            ]

    # fm_w^T packed as [128, kt*64]: FW[p, kt*64+h] = fm_w[h, kt*128+p]
    FW = (
        fm_w.T.reshape(KT, P, HEADS).transpose(1, 0, 2).reshape(P, KT * HEADS)
        .astype(bf)
    )
    # 0.5 * sum_r V^2, same packing
    SQ = (
        (0.5 * (fm_V.astype(np.float64) ** 2).sum(axis=1))
        .T.reshape(KT, P, HEADS).transpose(1, 0, 2).reshape(P, KT * HEADS)
        .astype(bf)
    )
    common = {
        "W1": np.ascontiguousarray(W1.astype(bf)),
        "W2": np.ascontiguousarray(W2.astype(bf)),
        "W3": np.ascontiguousarray(W3.astype(bf)),
        "B1": np.ascontiguousarray(b1.astype(f32).reshape(JT, P).T),
        "B2": np.ascontiguousarray(b2.astype(f32).reshape(JT, P).T),
        "B3": np.ascontiguousarray(b3.astype(f32).reshape(JT, P).T),
        "VTP": np.ascontiguousarray(VTP),
        "FWSQ": np.ascontiguousarray(np.concatenate([FW, SQ], axis=1)),
        "W0C": np.ascontiguousarray(
            np.tile((-fm_w0.astype(np.float64) / P)[None, :], (P, 1)).astype(bf)
        ),
    }

    in_maps = []
    xb = x.astype(bf)
    for c in range(NCORES):
        m = dict(common)
        m["xT"] = np.ascontiguousarray(xb[c * BC : (c + 1) * BC, :].T)
        in_maps.append(m)
    return in_maps


def kernel(x, W1, b1, W2, b2, W3, b3, fm_w0, fm_w, fm_V):
    # Host prep is plain numpy; coerce eagerly in case inputs are jax arrays.
    x, W1, b1, W2, b2, W3, b3, fm_w0, fm_w, fm_V = (
        np.asarray(a) for a in (x, W1, b1, W2, b2, W3, b3, fm_w0, fm_w, fm_V)
    )
    nc = _get_nc()
    in_maps = _prep_host(x, W1, b1, W2, b2, W3, b3, fm_w0, fm_w, fm_V)
    import os
    trace = bool(int(os.environ.get("KERNEL_TRACE", "0")))
    last_err = None
    for _attempt in range(3):
        try:
            res = bass_utils.run_bass_kernel_spmd(
                nc, in_maps, core_ids=list(range(NCORES)), trace=trace,
            )
            outs = [np.asarray(res.results[c]["out"]) for c in range(NCORES)]
            break
        except Exception as e:  # transient device faults (NRT unrecoverable)
            last_err = e
    else:
        raise last_err
    _CACHE["last_results"] = res
    full = np.concatenate(outs, axis=0)          # [B, HEADS]
    return np.ascontiguousarray(full.T).astype(np.float32)  # [HEADS, B]


# revision 15
# speedup vs baseline: 1.0006x; 1.0006x over previous
"""Trainium2 Bass kernel for NNBlendFM: 3-layer tanh MLP embedder + 64-head
rank-16 factorization machine, data-parallel over batch across 8 NeuronCores.

Math (per batch row b, head h):
    h = tanh(tanh(tanh(x W1 + b1) W2 + b2) W3 + b3)          # [B, 2048]
    lin[b,h]  = h . fm_w[h]
    vx[b,h,r] = h . fm_V[h,r]
    diag[b,h] = (h*h) . (sum_r fm_V[h,r]^2)
    out[h,b]  = fm_w0[h] + lin + 0.5*(sum_r vx^2 - diag)

All matmul operands are bf16 (fp32 PSUM accumulation).  fp16 was measured
~20% SLOWER per matmul row on this PE despite the cost model's 1.0
cycles/row claim, so bf16 it is; its 4.1e-3 rel err clears the 2e-2 gate.

Schedule notes (from trace analysis):
  - The PE stream is ~737.8K moving rows ~= 307us at 2.4 GHz; everything else
    is head/tail.  DMA descriptors drain in ~global issue order, so the head
    set (b1, x, W1) is issued first, and the 17MB W2/W3/VT stream follows on
    the SAME sync queue so it cannot starve the layer-1 inputs.
  - One 4KB-slot weight pool (32 bufs): alloc order warm, W1, W2, W3, VT
    (k-pair packed), FWSQ.  The ring makes W3's last five tiles alias the
    warm-up/W1 buffers (free ~35us) instead of W2 (free only at L2 end),
    and VT alias W2 (first read ~266us) — no more W3-wait stalls in L3.
  - PE warm-up matmuls on a vector-memset tile ramp HAM (1.2 -> 2.4 GHz)
    during the DMA head.
"""

import numpy as np
import ml_dtypes

import concourse.tile as tile
from concourse import bacc, mybir
from concourse import bass_utils

BF16 = mybir.dt.bfloat16
F32 = mybir.dt.float32
AF = mybir.ActivationFunctionType
ALU = mybir.AluOpType

P = 128
IN, HID, HEADS, RANK = 512, 2048, 64, 16
B = 8192
NCORES = 8
BC = B // NCORES            # 1024 batch rows per core
KT1 = IN // P               # 4  k-tiles, layer 1
KT = HID // P               # 16 k-tiles, layers 2/3 + FM
JT = HID // P               # 16 output-feature tiles per layer
NB = 512                    # matmul moving free-dim (one PSUM bank)
NBC = BC // NB              # 2 batch column chunks
BT = BC // P                # 8 batch tiles in FM stage
HR = HEADS * RANK           # 1024 vx columns

_CACHE = {}


def _build_module():
    nc = bacc.Bacc(
        "TRN2", target_bir_lowering=False, debug=False, num_devices=NCORES
    )
    dt = nc.dram_tensor
    xT = dt("xT", [IN, BC], BF16, kind="ExternalInput").ap()
    W1 = dt("W1", [IN, HID], BF16, kind="ExternalInput").ap()
    W2 = dt("W2", [HID, HID], BF16, kind="ExternalInput").ap()
    W3 = dt("W3", [HID, HID], BF16, kind="ExternalInput").ap()
    B1 = dt("B1", [P, JT], F32, kind="ExternalInput").ap()
    B2 = dt("B2", [P, JT], F32, kind="ExternalInput").ap()
    B3 = dt("B3", [P, JT], F32, kind="ExternalInput").ap()
    # V^T packed as k-pairs: VTP[p, kp*2*HR + i*HR + hr] = VT[(2kp+i)*128+p, hr]
    VTP = dt("VTP", [P, (KT // 2) * 2 * HR], BF16, kind="ExternalInput").ap()
    # fm_w^T and 0.5*sum_r V^2 packed side by side in one [128, 2*KT*HEADS]
    FWSQ = dt("FWSQ", [P, 2 * KT * HEADS], BF16, kind="ExternalInput").ap()
    W0C = dt("W0C", [P, HEADS], BF16, kind="ExternalInput").ap()
    OUT = dt("out", [BC, HEADS], F32, kind="ExternalOutput").ap()

    with tile.TileContext(nc) as tc:
        with (
            tc.tile_pool(name="wpool", bufs=32) as wpool,   # 4KB slots
            tc.tile_pool(name="hpool", bufs=32) as hpool,   # 2KB slots
            tc.tile_pool(name="cpool", bufs=1) as cpool,
            tc.tile_pool(name="pp", bufs=8, space="PSUM") as pp,
            tc.tile_pool(name="epool", bufs=2) as epool,
            tc.tile_pool(name="spool", bufs=8) as spool,
            tc.tile_pool(name="opool", bufs=4) as opool,
        ):
            # --- wpool alloc order defines the alias ring (32 bufs):
            #   0: warm, 1-4: W1, 5-20: W2, 21-36: W3 (33-36 alias warm+W1),
            #   37-44: VT pairs (alias W2_0..7), 45: FWSQ (aliases W2_8).
            warm = wpool.tile([P, HID], BF16, tag="w", name="warm")
            nc.vector.memset(warm[:, 0:NB], 0.0)

            # DMA descriptors drain in roughly global issue order, so the
            # program-order of dma_starts IS the fabric priority.  The whole
            # head set goes on sync in exact need-order (b1, x/W1 pairs),
            # with x2/x3 on scalar in parallel to save issue latency.  The
            # heavy W2/W3/VT stream follows ON THE SAME sync queue so it
            # cannot get ahead of the layer-1 inputs.
            b1t = cpool.tile([P, JT], F32, tag="b1")
            nc.gpsimd.dma_start(b1t[:], B1)
            # x0 is column-split (the first matmuls need only batch chunk
            # c0), W1's kt0 row-block is split into four jt-group quarters
            # and kt1 into halves — arrival granularity tracks the psum-wave
            # need order so layer 1 starts ~9.5us on a still-ramping DMA.
            x0h = [
                hpool.tile([P, NB], BF16, tag="h", name=f"x0h{c}")
                for c in range(2)
            ]
            xt = [None] + [
                hpool.tile([P, BC], BF16, tag="h", name=f"xt{k}")
                for k in range(1, KT1)
            ]
            w1q0 = [
                wpool.tile([P, NB], BF16, tag="w", name=f"w1q0_{q}")
                for q in range(4)
            ]
            w1h1 = [
                wpool.tile([P, HID // 2], BF16, tag="w", name=f"w1h1_{h}")
                for h in range(2)
            ]
            w1t = [None, None] + [
                wpool.tile([P, HID], BF16, tag="w", name=f"w1_{k}")
                for k in range(2, KT1)
            ]
            nc.sync.dma_start(x0h[0][:], xT[0:P, 0:NB])
            nc.sync.dma_start(x0h[1][:], xT[0:P, NB:BC])
            for k in range(1, KT1):
                nc.sync.dma_start(xt[k][:], xT[k * P : (k + 1) * P, :])
            for q in range(4):
                nc.scalar.dma_start(
                    w1q0[q][:], W1[0:P, q * NB : (q + 1) * NB]
                )
            for h in range(2):
                nc.scalar.dma_start(
                    w1h1[h][:],
                    W1[P : 2 * P, h * (HID // 2) : (h + 1) * (HID // 2)],
                )
            for k in range(2, KT1):
                nc.scalar.dma_start(w1t[k][:], W1[k * P : (k + 1) * P, :])

            def l1_lhsT(kt, jt):
                if kt == 0:
                    return w1q0[jt // 4][:, (jt % 4) * P : (jt % 4 + 1) * P]
                if kt == 1:
                    half, j = (w1h1[0], jt) if jt < 8 else (w1h1[1], jt - 8)
                    return half[:, j * P : (j + 1) * P]
                return w1t[kt][:, jt * P : (jt + 1) * P]

            def l1_rhs(kt, c):
                if kt == 0:
                    return x0h[c][:]
                return xt[kt][:, c * NB : (c + 1) * NB]

            # PE warm-up: ramps the clock while the head DMAs fly.
            wu = pp.tile([P, NB], F32, tag="ps", name="warm")
            for _ in range(4):
                nc.tensor.matmul(
                    wu[:], warm[:, 0:P], warm[:, 0:NB], start=True, stop=True
                )

            # Non-critical small constants on gpsimd (tiny transfers).
            b2t = cpool.tile([P, JT], F32, tag="b2")
            nc.gpsimd.dma_start(b2t[:], B2)
            b3t = cpool.tile([P, JT], F32, tag="b3")
            nc.gpsimd.dma_start(b3t[:], B3)
            # -w0/128 replicated; contracted against a ones column block so
            # the diag PSUM group finishes as (0.5*diag - w0).
            w0c = cpool.tile([P, HEADS], BF16, tag="w0c")
            nc.gpsimd.dma_start(w0c[:], W0C)
            onest = cpool.tile([P, P], BF16, tag="ones")
            nc.vector.memset(onest[:], 1.0)

            # Heavy weights in strict priority order on sync's ring.
            def load_w(dram, ktiles, name):
                ts = []
                for k in range(ktiles):
                    w_k = wpool.tile([P, HID], BF16, tag="w", name=f"{name}_{k}")
                    nc.sync.dma_start(w_k[:], dram[k * P : (k + 1) * P, :])
                    ts.append(w_k)
                return ts

            w2t = load_w(W2, KT, "w2")
            w3t = load_w(W3, KT, "w3")
            vtp = []
            for kp in range(KT // 2):
                v_k = wpool.tile([P, 2 * HR], BF16, tag="w", name=f"vt{kp}")
                nc.sync.dma_start(
                    v_k[:], VTP[:, kp * 2 * HR : (kp + 1) * 2 * HR]
                )
                vtp.append(v_k)

            def vtt(kt):                       # [128, HR] view of V^T k-tile
                return vtp[kt // 2][:, (kt % 2) * HR : (kt % 2 + 1) * HR]

            fwsq = wpool.tile([P, 2 * KT * HEADS], BF16, tag="w", name="fwsq")
            nc.sync.dma_start(fwsq[:], FWSQ)
            fwt = fwsq[:, 0 : KT * HEADS]
            sqt = fwsq[:, KT * HEADS : 2 * KT * HEADS]

            # --- embedder layers ------------------------------------------
            def layer(rhs_fn, lhsT_fn, bias_t, ktiles, name, rot):
                h_out = []
                for jt in range(JT):
                    ps = []
                    for c in range(NBC):
                        ps_c = pp.tile([P, NB], F32, tag="ps", name=f"{name}ps{jt}_{c}")
                        ps.append(ps_c)
                    # Rotate the accumulation start by jt (mod rot) so every
                    # jt can begin with an already-arrived k-tile while later
                    # tiles stream in.
                    kts = [(kt + jt) % rot for kt in range(rot)] + list(
                        range(rot, ktiles)
                    )
                    for i, kt in enumerate(kts):
                        lhsT = lhsT_fn(kt, jt)
                        for c in range(NBC):
                            nc.tensor.matmul(
                                ps[c][:],
                                lhsT,
                                rhs_fn(kt, c),
                                start=(i == 0),
                                stop=(i == ktiles - 1),
                            )
                    ht = hpool.tile([P, BC], BF16, tag="h", name=f"{name}h{jt}")
                    for c in range(NBC):
                        nc.scalar.activation(
                            ht[:, c * NB : (c + 1) * NB],
                            ps[c][:],
                            AF.Tanh,
                            bias=bias_t[:, jt : jt + 1],
                        )
                    h_out.append(ht)
                return h_out

            def w_lhsT(tiles):
                return lambda kt, jt: tiles[kt][:, jt * P : (jt + 1) * P]

            def h_rhs(tiles):
                return lambda kt, c: tiles[kt][:, c * NB : (c + 1) * NB]

            h1 = layer(l1_rhs, l1_lhsT, b1t, KT1, "l1", rot=1)
            h2 = layer(h_rhs(h1), w_lhsT(w2t), b2t, KT, "l2", rot=KT)
            h3 = layer(h_rhs(h2), w_lhsT(w3t), b3t, KT, "l3", rot=KT)

            # --- h3 squared (stationary operand for the diag matmuls) -----
            h3sq = []
            for k in range(KT):
                sq_k = hpool.tile([P, BC], BF16, tag="h", name=f"h3sq{k}")
                nc.vector.tensor_mul(sq_k[:], h3[k][:], h3[k][:])
                h3sq.append(sq_k)

            # --- FM stage: per 128-row batch tile -------------------------
            def fm_phase_a(bt):
                """vx = h V^T (1024 cols) and lin = h fm_w^T (64 cols)."""
                vx0 = pp.tile([P, NB], F32, tag="ps", name=f"vx0_{bt}")
                vx1 = pp.tile([P, NB], F32, tag="ps", name=f"vx1_{bt}")
                lw = pp.tile([P, NB], F32, tag="ps", name=f"lw_{bt}")
                bsl = slice(bt * P, (bt + 1) * P)
                for kt in range(KT):
                    lhsT = h3[kt][:, bsl]
                    vt_k = vtt(kt)
                    nc.tensor.matmul(
                        vx0[:], lhsT, vt_k[:, 0:NB],
                        start=(kt == 0), stop=(kt == KT - 1),
                    )
                    nc.tensor.matmul(
                        vx1[:], lhsT, vt_k[:, NB:HR],
                        start=(kt == 0), stop=(kt == KT - 1),
                    )
                    nc.tensor.matmul(
                        lw[:, 0:HEADS], lhsT,
                        fwt[:, kt * HEADS : (kt + 1) * HEADS],
                        start=(kt == 0), stop=(kt == KT - 1),
                    )
                return vx0, vx1, lw

            def fm_phase_b(bt):
                """diag = (h*h) . (0.5 * sum_r V^2), already scaled by 0.5."""
                dg = pp.tile([P, NB], F32, tag="ps", name=f"dg_{bt}")
                bsl = slice(bt * P, (bt + 1) * P)
                for kt in range(KT):
                    nc.tensor.matmul(
                        dg[:, 0:HEADS],
                        h3sq[kt][:, bsl],
                        sqt[:, kt * HEADS : (kt + 1) * HEADS],
                        start=(kt == 0), stop=False,
                    )
                nc.tensor.matmul(
                    dg[:, 0:HEADS], onest[:], w0c[:], start=False, stop=True,
                )
                return dg

            def fm_square_reduce(bt, vx0, vx1):
                """Emitted right after phase A: overlaps later bt's matmuls.
                Each 512-wide half squares then reduces independently so the
                two chains pipeline across ACT and DVE."""
                vx2 = epool.tile([P, HR], BF16, tag="e", name=f"vx2_{bt}")
                sumv = spool.tile([P, HEADS], F32, tag="s", name=f"sumv_{bt}")
                for c, vxh in ((0, vx0), (1, vx1)):
                    nc.scalar.activation(vx2[:, c * NB : (c + 1) * NB], vxh[:], AF.Square)
                    nc.vector.reduce_sum(
                        sumv[:, c * (HEADS // 2) : (c + 1) * (HEADS // 2)],
                        vx2[:, c * NB : (c + 1) * NB].rearrange(
                            "p (h r) -> p h r", r=RANK
                        ),
                        axis=mybir.AxisListType.X,
                    )
                return sumv

            def fm_combine(bt, sumv, lw, dg):
                # q = 0.5*sumv - diag_half
                q = spool.tile([P, HEADS], F32, tag="s", name=f"q_{bt}")
                nc.vector.scalar_tensor_tensor(
                    q[:], sumv[:], 0.5, dg[:, 0:HEADS],
                    op0=ALU.mult, op1=ALU.subtract,
                )
                ot = opool.tile([P, HEADS], F32, tag="o", name=f"ot_{bt}")
                nc.vector.tensor_add(ot[:], q[:], lw[:, 0:HEADS])
                nc.sync.dma_start(OUT[bt * P : (bt + 1) * P, :], ot[:])

            # Stagger: A(0), A(1), B(0), C(0), A(2), B(1), C(1), ...
            pend = []  # (bt, sumv, lw)
            for bt in range(BT):
                vx0, vx1, lw = fm_phase_a(bt)
                sumv = fm_square_reduce(bt, vx0, vx1)
                pend.append((bt, sumv, lw))
                if len(pend) == 2:
                    obt, osumv, olw = pend.pop(0)
                    dg = fm_phase_b(obt)
                    fm_combine(obt, osumv, olw, dg)
            while pend:
                obt, osumv, olw = pend.pop(0)
                dg = fm_phase_b(obt)
                fm_combine(obt, osumv, olw, dg)

    nc.compile()
    return nc


def _get_nc():
    if "nc" not in _CACHE:
        _CACHE["nc"] = _build_module()
    return _CACHE["nc"]


def _prep_host(x, W1, b1, W2, b2, W3, b3, fm_w0, fm_w, fm_V):
    """Host-side layout prep: bf16 casts, transposes, per-head V reductions."""
    bf = ml_dtypes.bfloat16
    f32 = np.float32

    # V^T: [2048, heads*rank], col hr = h*RANK + r; then k-pair packed
    VT = fm_V.reshape(HEADS * RANK, HID).T.astype(bf)          # [2048, 1024]
    VTP = np.empty((P, (KT // 2) * 2 * HR), dtype=bf)
    for kp in range(KT // 2):
        for i in range(2):
            VTP[:, kp * 2 * HR + i * HR : kp * 2 * HR + (i + 1) * HR] = VT[
                (2 * kp + i) * P : (2 * kp + i + 1) * P, :
            ]

    # fm_w^T packed as [128, kt*64]: FW[p, kt*64+h] = fm_w[h, kt*128+p]
    FW = (
        fm_w.T.reshape(KT, P, HEADS).transpose(1, 0, 2).reshape(P, KT * HEADS)
        .astype(bf)
    )
    # 0.5 * sum_r V^2, same packing
    SQ = (
        (0.5 * (fm_V.astype(np.float64) ** 2).sum(axis=1))
        .T.reshape(KT, P, HEADS).transpose(1, 0, 2).reshape(P, KT * HEADS)
        .astype(bf)
    )
    common = {
        "W1": np.ascontiguousarray(W1.astype(bf)),
        "W2": np.ascontiguousarray(W2.astype(bf)),
        "W3": np.ascontiguousarray(W3.astype(bf)),
        "B1": np.ascontiguousarray(b1.astype(f32).reshape(JT, P).T),
        "B2": np.ascontiguousarray(b2.astype(f32).reshape(JT, P).T),
        "B3": np.ascontiguousarray(b3.astype(f32).reshape(JT, P).T),
        "VTP": np.ascontiguousarray(VTP),
        "FWSQ": np.ascontiguousarray(np.concatenate([FW, SQ], axis=1)),
        "W0C": np.ascontiguousarray(
            np.tile((-fm_w0.astype(np.float64) / P)[None, :], (P, 1)).astype(bf)
        ),
    }

    in_maps = []
    xb = x.astype(bf)
    for c in range(NCORES):
        m = dict(common)
        m["xT"] = np.ascontiguousarray(xb[c * BC : (c + 1) * BC, :].T)
        in_maps.append(m)
    return in_maps


def kernel(x, W1, b1, W2, b2, W3, b3, fm_w0, fm_w, fm_V):
    # Host prep is plain numpy; coerce eagerly in case inputs are jax arrays.
    x, W1, b1, W2, b2, W3, b3, fm_w0, fm_w, fm_V = (
        np.asarray(a) for a in (x, W1, b1, W2, b2, W3, b3, fm_w0, fm_w, fm_V)
    )
    nc = _get_nc()
    in_maps = _prep_host(x, W1, b1, W2, b2, W3, b3, fm_w0, fm_w, fm_V)
    import os
    trace = bool(int(os.environ.get("KERNEL_TRACE", "0")))
    last_err = None
    for _attempt in range(3):
        try:
            res = bass_utils.run_bass_kernel_spmd(
                nc, in_maps, core_ids=list(range(NCORES)), trace=trace,
            )
            outs = [np.asarray(res.results[c]["out"]) for c in range(NCORES)]
            break
        except Exception as e:  # transient device faults (NRT unrecoverable)
            last_err = e
    else:
        raise last_err
    _CACHE["last_results"] = res
    full = np.concatenate(outs, axis=0)          # [B, HEADS]
    return np.ascontiguousarray(full.T).astype(np.float32)  # [HEADS, B]


# revision 16
# speedup vs baseline: 1.0072x; 1.0066x over previous
"""Trainium2 Bass kernel for NNBlendFM: 3-layer tanh MLP embedder + 64-head
rank-16 factorization machine, data-parallel over batch across 8 NeuronCores.

Math (per batch row b, head h):
    h = tanh(tanh(tanh(x W1 + b1) W2 + b2) W3 + b3)          # [B, 2048]
    lin[b,h]  = h . fm_w[h]
    vx[b,h,r] = h . fm_V[h,r]
    diag[b,h] = (h*h) . (sum_r fm_V[h,r]^2)
    out[h,b]  = fm_w0[h] + lin + 0.5*(sum_r vx^2 - diag)

All matmul operands are bf16 (fp32 PSUM accumulation).  fp16 was measured
~20% SLOWER per matmul row on this PE despite the cost model's 1.0
cycles/row claim, so bf16 it is; its 4.1e-3 rel err clears the 2e-2 gate.

Schedule notes (from trace analysis):
  - The PE stream is ~737.8K moving rows ~= 307us at 2.4 GHz; everything else
    is head/tail.  DMA descriptors drain in ~global issue order, so the head
    set (b1, x, W1) is issued first, and the 17MB W2/W3/VT stream follows on
    the SAME sync queue so it cannot starve the layer-1 inputs.
  - One 4KB-slot weight pool (32 bufs): alloc order warm, W1, W2, W3, VT
    (k-pair packed), FWSQ.  The ring makes W3's last five tiles alias the
    warm-up/W1 buffers (free ~35us) instead of W2 (free only at L2 end),
    and VT alias W2 (first read ~266us) — no more W3-wait stalls in L3.
  - PE warm-up matmuls on a vector-memset tile ramp HAM (1.2 -> 2.4 GHz)
    during the DMA head.
"""

import numpy as np
import ml_dtypes

import concourse.tile as tile
from concourse import bacc, mybir
from concourse import bass_utils

BF16 = mybir.dt.bfloat16
F32 = mybir.dt.float32
AF = mybir.ActivationFunctionType
ALU = mybir.AluOpType

P = 128
IN, HID, HEADS, RANK = 512, 2048, 64, 16
B = 8192
NCORES = 8
BC = B // NCORES            # 1024 batch rows per core
KT1 = IN // P               # 4  k-tiles, layer 1
KT = HID // P               # 16 k-tiles, layers 2/3 + FM
JT = HID // P               # 16 output-feature tiles per layer
NB = 512                    # matmul moving free-dim (one PSUM bank)
NBC = BC // NB              # 2 batch column chunks
BT = BC // P                # 8 batch tiles in FM stage
HR = HEADS * RANK           # 1024 vx columns

_CACHE = {}


def _build_module():
    nc = bacc.Bacc(
        "TRN2", target_bir_lowering=False, debug=False, num_devices=NCORES
    )
    dt = nc.dram_tensor
    xT = dt("xT", [IN, BC], BF16, kind="ExternalInput").ap()
    W1 = dt("W1", [IN, HID], BF16, kind="ExternalInput").ap()
    W2 = dt("W2", [HID, HID], BF16, kind="ExternalInput").ap()
    W3 = dt("W3", [HID, HID], BF16, kind="ExternalInput").ap()
    B1 = dt("B1", [P, JT], F32, kind="ExternalInput").ap()
    B2 = dt("B2", [P, JT], F32, kind="ExternalInput").ap()
    B3 = dt("B3", [P, JT], F32, kind="ExternalInput").ap()
    # V^T packed as k-pairs: VTP[p, kp*2*HR + i*HR + hr] = VT[(2kp+i)*128+p, hr]
    VTP = dt("VTP", [P, (KT // 2) * 2 * HR], BF16, kind="ExternalInput").ap()
    # fm_w^T and 0.5*sum_r V^2 packed side by side in one [128, 2*KT*HEADS]
    FWSQ = dt("FWSQ", [P, 2 * KT * HEADS], BF16, kind="ExternalInput").ap()
    W0C = dt("W0C", [P, HEADS], BF16, kind="ExternalInput").ap()
    OUT = dt("out", [BC, HEADS], F32, kind="ExternalOutput").ap()

    with tile.TileContext(nc) as tc:
        with (
            tc.tile_pool(name="wpool", bufs=32) as wpool,   # 4KB slots
            tc.tile_pool(name="hpool", bufs=32) as hpool,   # 2KB slots
            tc.tile_pool(name="cpool", bufs=1) as cpool,
            tc.tile_pool(name="pp", bufs=8, space="PSUM") as pp,
            tc.tile_pool(name="epool", bufs=2) as epool,
            tc.tile_pool(name="spool", bufs=8) as spool,
            tc.tile_pool(name="opool", bufs=4) as opool,
        ):
            # --- wpool alloc order defines the alias ring (32 bufs):
            #   0: warm, 1-4: W1, 5-20: W2, 21-36: W3 (33-36 alias warm+W1),
            #   37-44: VT pairs (alias W2_0..7), 45: FWSQ (aliases W2_8).
            warm = wpool.tile([P, HID], BF16, tag="w", name="warm")
            nc.vector.memset(warm[:, 0:NB], 0.0)

            # DMA descriptors drain in roughly global issue order, so the
            # program-order of dma_starts IS the fabric priority.  The whole
            # head set goes on sync in exact need-order (b1, x/W1 pairs),
            # with x2/x3 on scalar in parallel to save issue latency.  The
            # heavy W2/W3/VT stream follows ON THE SAME sync queue so it
            # cannot get ahead of the layer-1 inputs.
            b1t = cpool.tile([P, JT], F32, tag="b1")
            nc.gpsimd.dma_start(b1t[:], B1)
            # x0 is column-split (the first matmuls need only batch chunk
            # c0), W1's kt0 row-block is split into four jt-group quarters
            # and kt1 into halves — arrival granularity tracks the psum-wave
            # need order so layer 1 starts ~9.5us on a still-ramping DMA.
            x0h = [
                hpool.tile([P, NB], BF16, tag="h", name=f"x0h{c}")
                for c in range(2)
            ]
            xt = [None] + [
                hpool.tile([P, BC], BF16, tag="h", name=f"xt{k}")
                for k in range(1, KT1)
            ]
            w1q0 = [
                wpool.tile([P, NB], BF16, tag="w", name=f"w1q0_{q}")
                for q in range(4)
            ]
            w1h1 = [
                wpool.tile([P, HID // 2], BF16, tag="w", name=f"w1h1_{h}")
                for h in range(2)
            ]
            w1t = [None, None] + [
                wpool.tile([P, HID], BF16, tag="w", name=f"w1_{k}")
                for k in range(2, KT1)
            ]
            nc.sync.dma_start(x0h[0][:], xT[0:P, 0:NB])
            nc.sync.dma_start(x0h[1][:], xT[0:P, NB:BC])
            for k in range(1, KT1):
                nc.sync.dma_start(xt[k][:], xT[k * P : (k + 1) * P, :])
            for q in range(4):
                nc.scalar.dma_start(
                    w1q0[q][:], W1[0:P, q * NB : (q + 1) * NB]
                )
            for h in range(2):
                nc.scalar.dma_start(
                    w1h1[h][:],
                    W1[P : 2 * P, h * (HID // 2) : (h + 1) * (HID // 2)],
                )
            nc.scalar.dma_start(w1t[2][:], W1[2 * P : 3 * P, :])
            nc.sync.dma_start(w1t[3][:], W1[3 * P : 4 * P, :])

            def l1_lhsT(kt, jt):
                if kt == 0:
                    return w1q0[jt // 4][:, (jt % 4) * P : (jt % 4 + 1) * P]
                if kt == 1:
                    half, j = (w1h1[0], jt) if jt < 8 else (w1h1[1], jt - 8)
                    return half[:, j * P : (j + 1) * P]
                return w1t[kt][:, jt * P : (jt + 1) * P]

            def l1_rhs(kt, c):
                if kt == 0:
                    return x0h[c][:]
                return xt[kt][:, c * NB : (c + 1) * NB]

            # PE warm-up: ramps the clock while the head DMAs fly.
            wu = pp.tile([P, NB], F32, tag="ps", name="warm")
            for _ in range(4):
                nc.tensor.matmul(
                    wu[:], warm[:, 0:P], warm[:, 0:NB], start=True, stop=True
                )

            # Non-critical small constants on gpsimd (tiny transfers).
            b2t = cpool.tile([P, JT], F32, tag="b2")
            nc.gpsimd.dma_start(b2t[:], B2)
            b3t = cpool.tile([P, JT], F32, tag="b3")
            nc.gpsimd.dma_start(b3t[:], B3)
            # -w0/128 replicated; contracted against a ones column block so
            # the diag PSUM group finishes as (0.5*diag - w0).
            w0c = cpool.tile([P, HEADS], BF16, tag="w0c")
            nc.gpsimd.dma_start(w0c[:], W0C)
            onest = cpool.tile([P, P], BF16, tag="ones")
            nc.vector.memset(onest[:], 1.0)

            # Heavy weights in strict priority order on sync's ring.
            def load_w(dram, ktiles, name):
                ts = []
                for k in range(ktiles):
                    w_k = wpool.tile([P, HID], BF16, tag="w", name=f"{name}_{k}")
                    nc.sync.dma_start(w_k[:], dram[k * P : (k + 1) * P, :])
                    ts.append(w_k)
                return ts

            w2t = load_w(W2, KT, "w2")
            w3t = load_w(W3, KT, "w3")
            vtp = []
            for kp in range(KT // 2):
                v_k = wpool.tile([P, 2 * HR], BF16, tag="w", name=f"vt{kp}")
                nc.sync.dma_start(
                    v_k[:], VTP[:, kp * 2 * HR : (kp + 1) * 2 * HR]
                )
                vtp.append(v_k)

            def vtt(kt):                       # [128, HR] view of V^T k-tile
                return vtp[kt // 2][:, (kt % 2) * HR : (kt % 2 + 1) * HR]

            fwsq = wpool.tile([P, 2 * KT * HEADS], BF16, tag="w", name="fwsq")
            nc.sync.dma_start(fwsq[:], FWSQ)
            fwt = fwsq[:, 0 : KT * HEADS]
            sqt = fwsq[:, KT * HEADS : 2 * KT * HEADS]

            # --- embedder layers ------------------------------------------
            def layer(rhs_fn, lhsT_fn, bias_t, ktiles, name, rot):
                h_out = []
                for jt in range(JT):
                    ps = []
                    for c in range(NBC):
                        ps_c = pp.tile([P, NB], F32, tag="ps", name=f"{name}ps{jt}_{c}")
                        ps.append(ps_c)
                    # Rotate the accumulation start by jt (mod rot) so every
                    # jt can begin with an already-arrived k-tile while later
                    # tiles stream in.
                    kts = [(kt + jt) % rot for kt in range(rot)] + list(
                        range(rot, ktiles)
                    )
                    for i, kt in enumerate(kts):
                        lhsT = lhsT_fn(kt, jt)
                        for c in range(NBC):
                            nc.tensor.matmul(
                                ps[c][:],
                                lhsT,
                                rhs_fn(kt, c),
                                start=(i == 0),
                                stop=(i == ktiles - 1),
                            )
                    ht = hpool.tile([P, BC], BF16, tag="h", name=f"{name}h{jt}")
                    for c in range(NBC):
                        nc.scalar.activation(
                            ht[:, c * NB : (c + 1) * NB],
                            ps[c][:],
                            AF.Tanh,
                            bias=bias_t[:, jt : jt + 1],
                        )
                    h_out.append(ht)
                return h_out

            def w_lhsT(tiles):
                return lambda kt, jt: tiles[kt][:, jt * P : (jt + 1) * P]

            def h_rhs(tiles):
                return lambda kt, c: tiles[kt][:, c * NB : (c + 1) * NB]

            h1 = layer(l1_rhs, l1_lhsT, b1t, KT1, "l1", rot=1)
            h2 = layer(h_rhs(h1), w_lhsT(w2t), b2t, KT, "l2", rot=KT)
            h3 = layer(h_rhs(h2), w_lhsT(w3t), b3t, KT, "l3", rot=KT)

            # --- h3 squared (stationary operand for the diag matmuls) -----
            h3sq = []
            for k in range(KT):
                sq_k = hpool.tile([P, BC], BF16, tag="h", name=f"h3sq{k}")
                nc.vector.tensor_mul(sq_k[:], h3[k][:], h3[k][:])
                h3sq.append(sq_k)

            # --- FM stage: per 128-row batch tile -------------------------
            def fm_phase_a(bt):
                """vx = h V^T (1024 cols) and lin = h fm_w^T (64 cols)."""
                vx0 = pp.tile([P, NB], F32, tag="ps", name=f"vx0_{bt}")
                vx1 = pp.tile([P, NB], F32, tag="ps", name=f"vx1_{bt}")
                lw = pp.tile([P, NB], F32, tag="ps", name=f"lw_{bt}")
                bsl = slice(bt * P, (bt + 1) * P)
                for kt in range(KT):
                    lhsT = h3[kt][:, bsl]
                    vt_k = vtt(kt)
                    nc.tensor.matmul(
                        vx0[:], lhsT, vt_k[:, 0:NB],
                        start=(kt == 0), stop=(kt == KT - 1),
                    )
                    nc.tensor.matmul(
                        vx1[:], lhsT, vt_k[:, NB:HR],
                        start=(kt == 0), stop=(kt == KT - 1),
                    )
                    nc.tensor.matmul(
                        lw[:, 0:HEADS], lhsT,
                        fwt[:, kt * HEADS : (kt + 1) * HEADS],
                        start=(kt == 0), stop=(kt == KT - 1),
                    )
                return vx0, vx1, lw

            def fm_phase_b(bt):
                """diag = (h*h) . (0.5 * sum_r V^2), already scaled by 0.5."""
                dg = pp.tile([P, NB], F32, tag="ps", name=f"dg_{bt}")
                bsl = slice(bt * P, (bt + 1) * P)
                for kt in range(KT):
                    nc.tensor.matmul(
                        dg[:, 0:HEADS],
                        h3sq[kt][:, bsl],
                        sqt[:, kt * HEADS : (kt + 1) * HEADS],
                        start=(kt == 0), stop=False,
                    )
                nc.tensor.matmul(
                    dg[:, 0:HEADS], onest[:], w0c[:], start=False, stop=True,
                )
                return dg

            def fm_square_reduce(bt, vx0, vx1):
                """Emitted right after phase A: overlaps later bt's matmuls.
                Each 512-wide half squares then reduces independently so the
                two chains pipeline across ACT and DVE."""
                vx2 = epool.tile([P, HR], BF16, tag="e", name=f"vx2_{bt}")
                sumv = spool.tile([P, HEADS], F32, tag="s", name=f"sumv_{bt}")
                for c, vxh in ((0, vx0), (1, vx1)):
                    nc.scalar.activation(vx2[:, c * NB : (c + 1) * NB], vxh[:], AF.Square)
                    nc.vector.reduce_sum(
                        sumv[:, c * (HEADS // 2) : (c + 1) * (HEADS // 2)],
                        vx2[:, c * NB : (c + 1) * NB].rearrange(
                            "p (h r) -> p h r", r=RANK
                        ),
                        axis=mybir.AxisListType.X,
                    )
                return sumv

            def fm_combine(bt, sumv, lw, dg):
                # q = 0.5*sumv - diag_half
                q = spool.tile([P, HEADS], F32, tag="s", name=f"q_{bt}")
                nc.vector.scalar_tensor_tensor(
                    q[:], sumv[:], 0.5, dg[:, 0:HEADS],
                    op0=ALU.mult, op1=ALU.subtract,
                )
                ot = opool.tile([P, HEADS], F32, tag="o", name=f"ot_{bt}")
                nc.vector.tensor_add(ot[:], q[:], lw[:, 0:HEADS])
                nc.sync.dma_start(OUT[bt * P : (bt + 1) * P, :], ot[:])

            # Stagger: A(0), A(1), B(0), C(0), A(2), B(1), C(1), ...
            pend = []  # (bt, sumv, lw)
            for bt in range(BT):
                vx0, vx1, lw = fm_phase_a(bt)
                sumv = fm_square_reduce(bt, vx0, vx1)
                pend.append((bt, sumv, lw))
                if len(pend) == 2:
                    obt, osumv, olw = pend.pop(0)
                    dg = fm_phase_b(obt)
                    fm_combine(obt, osumv, olw, dg)
            while pend:
                obt, osumv, olw = pend.pop(0)
                dg = fm_phase_b(obt)
                fm_combine(obt, osumv, olw, dg)

    nc.compile()
    return nc


def _get_nc():
    if "nc" not in _CACHE:
        _CACHE["nc"] = _build_module()
    return _CACHE["nc"]


def _prep_host(x, W1, b1, W2, b2, W3, b3, fm_w0, fm_w, fm_V):
    """Host-side layout prep: bf16 casts, transposes, per-head V reductions."""
    bf = ml_dtypes.bfloat16
    f32 = np.float32

    # V^T: [2048, heads*rank], col hr = h*RANK + r; then k-pair packed
    VT = fm_V.reshape(HEADS * RANK, HID).T.astype(bf)          # [2048, 1024]
    VTP = np.empty((P, (KT // 2) * 2 * HR), dtype=bf)
    for kp in range(KT // 2):
        for i in range(2):
            VTP[:, kp * 2 * HR + i * HR : kp * 2 * HR + (i + 1) * HR] = VT[
                (2 * kp + i) * P : (2 * kp + i + 1) * P, :
            ]

    # fm_w^T packed as [128, kt*64]: FW[p, kt*64+h] = fm_w[h, kt*128+p]
    FW = (
        fm_w.T.reshape(KT, P, HEADS).transpose(1, 0, 2).reshape(P, KT * HEADS)
        .astype(bf)
    )
    # 0.5 * sum_r V^2, same packing
    SQ = (
        (0.5 * (fm_V.astype(np.float64) ** 2).sum(axis=1))
        .T.reshape(KT, P, HEADS).transpose(1, 0, 2).reshape(P, KT * HEADS)
        .astype(bf)
    )
    common = {
        "W1": np.ascontiguousarray(W1.astype(bf)),
        "W2": np.ascontiguousarray(W2.astype(bf)),
        "W3": np.ascontiguousarray(W3.astype(bf)),
        "B1": np.ascontiguousarray(b1.astype(f32).reshape(JT, P).T),
        "B2": np.ascontiguousarray(b2.astype(f32).reshape(JT, P).T),
        "B3": np.ascontiguousarray(b3.astype(f32).reshape(JT, P).T),
        "VTP": np.ascontiguousarray(VTP),
        "FWSQ": np.ascontiguousarray(np.concatenate([FW, SQ], axis=1)),
        "W0C": np.ascontiguousarray(
            np.tile((-fm_w0.astype(np.float64) / P)[None, :], (P, 1)).astype(bf)
        ),
    }

    in_maps = []
    xb = x.astype(bf)
    for c in range(NCORES):
        m = dict(common)
        m["xT"] = np.ascontiguousarray(xb[c * BC : (c + 1) * BC, :].T)
        in_maps.append(m)
    return in_maps


def kernel(x, W1, b1, W2, b2, W3, b3, fm_w0, fm_w, fm_V):
    # Host prep is plain numpy; coerce eagerly in case inputs are jax arrays.
    x, W1, b1, W2, b2, W3, b3, fm_w0, fm_w, fm_V = (
        np.asarray(a) for a in (x, W1, b1, W2, b2, W3, b3, fm_w0, fm_w, fm_V)
    )
    nc = _get_nc()
    in_maps = _prep_host(x, W1, b1, W2, b2, W3, b3, fm_w0, fm_w, fm_V)
    import os
    trace = bool(int(os.environ.get("KERNEL_TRACE", "0")))
    last_err = None
    for _attempt in range(3):
        try:
            res = bass_utils.run_bass_kernel_spmd(
                nc, in_maps, core_ids=list(range(NCORES)), trace=trace,
            )
            outs = [np.asarray(res.results[c]["out"]) for c in range(NCORES)]
            break
        except Exception as e:  # transient device faults (NRT unrecoverable)
            last_err = e
    else:
        raise last_err
    _CACHE["last_results"] = res
    full = np.concatenate(outs, axis=0)          # [B, HEADS]
    return np.ascontiguousarray(full.T).astype(np.float32)  # [HEADS, B]


# revision 17
# speedup vs baseline: 1.0090x; 1.0017x over previous
"""Trainium2 Bass kernel for NNBlendFM: 3-layer tanh MLP embedder + 64-head
rank-16 factorization machine, data-parallel over batch across 8 NeuronCores.

Math (per batch row b, head h):
    h = tanh(tanh(tanh(x W1 + b1) W2 + b2) W3 + b3)          # [B, 2048]
    lin[b,h]  = h . fm_w[h]
    vx[b,h,r] = h . fm_V[h,r]
    diag[b,h] = (h*h) . (sum_r fm_V[h,r]^2)
    out[h,b]  = fm_w0[h] + lin + 0.5*(sum_r vx^2 - diag)

All matmul operands are bf16 (fp32 PSUM accumulation).  fp16 was measured
~20% SLOWER per matmul row on this PE despite the cost model's 1.0
cycles/row claim, so bf16 it is; its 4.1e-3 rel err clears the 2e-2 gate.

Schedule notes (from trace analysis):
  - The PE stream is ~737.8K moving rows ~= 311us at the observed 2.37 GHz;
    everything else is head/tail.  The head splits x over sync and W1 over
    scalar in need order (kt0 in quarters), and the 17MB W2/W3/VT stream
    follows on the sync queue so it cannot starve the layer-1 inputs.
  - One 4KB-slot weight pool (32 bufs): alloc order warm, W1, W2, W3, VT
    (k-pair packed), FWSQ.  The ring makes W3's last five tiles alias the
    warm-up/W1 buffers (free ~35us) instead of W2 (free only at L2 end),
    and VT alias W2 (first read ~266us) — no more W3-wait stalls in L3.
  - PE warm-up matmuls on a vector-memset tile ramp HAM (1.2 -> 2.4 GHz)
    during the DMA head.
"""

import numpy as np
import ml_dtypes

import concourse.tile as tile
from concourse import bacc, mybir
from concourse import bass_utils

BF16 = mybir.dt.bfloat16
F32 = mybir.dt.float32
AF = mybir.ActivationFunctionType
ALU = mybir.AluOpType

P = 128
IN, HID, HEADS, RANK = 512, 2048, 64, 16
B = 8192
NCORES = 8
BC = B // NCORES            # 1024 batch rows per core
KT1 = IN // P               # 4  k-tiles, layer 1
KT = HID // P               # 16 k-tiles, layers 2/3 + FM
JT = HID // P               # 16 output-feature tiles per layer
NB = 512                    # matmul moving free-dim (one PSUM bank)
NBC = BC // NB              # 2 batch column chunks
BT = BC // P                # 8 batch tiles in FM stage
HR = HEADS * RANK           # 1024 vx columns

_CACHE = {}


def _build_module():
    nc = bacc.Bacc(
        "TRN2", target_bir_lowering=False, debug=False, num_devices=NCORES
    )
    dt = nc.dram_tensor
    xT = dt("xT", [IN, BC], BF16, kind="ExternalInput").ap()
    W1 = dt("W1", [IN, HID], BF16, kind="ExternalInput").ap()
    W2 = dt("W2", [HID, HID], BF16, kind="ExternalInput").ap()
    W3 = dt("W3", [HID, HID], BF16, kind="ExternalInput").ap()
    B1 = dt("B1", [P, JT], F32, kind="ExternalInput").ap()
    B2 = dt("B2", [P, JT], F32, kind="ExternalInput").ap()
    B3 = dt("B3", [P, JT], F32, kind="ExternalInput").ap()
    # V^T packed as k-pairs: VTP[p, kp*2*HR + i*HR + hr] = VT[(2kp+i)*128+p, hr]
    VTP = dt("VTP", [P, (KT // 2) * 2 * HR], BF16, kind="ExternalInput").ap()
    # fm_w^T and 0.5*sum_r V^2 packed side by side in one [128, 2*KT*HEADS]
    FWSQ = dt("FWSQ", [P, 2 * KT * HEADS], BF16, kind="ExternalInput").ap()
    W0C = dt("W0C", [P, HEADS], BF16, kind="ExternalInput").ap()
    OUT = dt("out", [BC, HEADS], F32, kind="ExternalOutput").ap()

    with tile.TileContext(nc) as tc:
        with (
            tc.tile_pool(name="wpool", bufs=32) as wpool,   # 4KB slots
            tc.tile_pool(name="hpool", bufs=32) as hpool,   # 2KB slots
            tc.tile_pool(name="cpool", bufs=1) as cpool,
            tc.tile_pool(name="pp", bufs=8, space="PSUM") as pp,
            tc.tile_pool(name="epool", bufs=2) as epool,
            tc.tile_pool(name="spool", bufs=8) as spool,
            tc.tile_pool(name="opool", bufs=4) as opool,
        ):
            # --- wpool alloc order defines the alias ring (32 bufs):
            #   warm, W1 pieces (7 tiles), W2 x16, W3 x16, VT pairs x8,
            #   FWSQ.  W3's last 9 tiles wrap onto the warm-up/W1 buffers
            #   (free by L1's end, ~38us) instead of W2 (free only at L2's
            #   end), and VT/FWSQ wrap onto W2 (first read ~266us) — so no
            #   weight DMA ever stalls the PE mid-layer.
            warm = wpool.tile([P, HID], BF16, tag="w", name="warm")
            nc.vector.memset(warm[:, 0:NB], 0.0)

            # Per-descriptor DMA throughput is only ~25-60 GB/s while the
            # fabric ramps, so the head runs x on sync and W1 on scalar
            # concurrently, in need order, and the heavy W2/W3/VT stream
            # follows on the sync queue so it cannot get ahead of them.
            b1t = cpool.tile([P, JT], F32, tag="b1")
            nc.gpsimd.dma_start(b1t[:], B1)
            # x0 is column-split (the first matmuls need only batch chunk
            # c0), W1's kt0 row-block is split into four jt-group quarters
            # and kt1 into halves — arrival granularity tracks the psum-wave
            # need order so layer 1 starts ~9.5us on a still-ramping DMA.
            x0h = [
                hpool.tile([P, NB], BF16, tag="h", name=f"x0h{c}")
                for c in range(2)
            ]
            xt = [None] + [
                hpool.tile([P, BC], BF16, tag="h", name=f"xt{k}")
                for k in range(1, KT1)
            ]
            w1q0 = [
                wpool.tile([P, NB], BF16, tag="w", name=f"w1q0_{q}")
                for q in range(4)
            ]
            w1h1 = [
                wpool.tile([P, HID // 2], BF16, tag="w", name=f"w1h1_{h}")
                for h in range(2)
            ]
            w1t = [None, None] + [
                wpool.tile([P, HID], BF16, tag="w", name=f"w1_{k}")
                for k in range(2, KT1)
            ]
            nc.sync.dma_start(x0h[0][:], xT[0:P, 0:NB])
            nc.sync.dma_start(x0h[1][:], xT[0:P, NB:BC])
            for k in range(1, KT1):
                nc.sync.dma_start(xt[k][:], xT[k * P : (k + 1) * P, :])
            for q in range(4):
                nc.scalar.dma_start(
                    w1q0[q][:], W1[0:P, q * NB : (q + 1) * NB]
                )
            for h in range(2):
                nc.scalar.dma_start(
                    w1h1[h][:],
                    W1[P : 2 * P, h * (HID // 2) : (h + 1) * (HID // 2)],
                )
            nc.scalar.dma_start(w1t[2][:], W1[2 * P : 3 * P, :])
            nc.sync.dma_start(w1t[3][:], W1[3 * P : 4 * P, :])

            def l1_lhsT(kt, jt):
                if kt == 0:
                    return w1q0[jt // 4][:, (jt % 4) * P : (jt % 4 + 1) * P]
                if kt == 1:
                    half, j = (w1h1[0], jt) if jt < 8 else (w1h1[1], jt - 8)
                    return half[:, j * P : (j + 1) * P]
                return w1t[kt][:, jt * P : (jt + 1) * P]

            def l1_rhs(kt, c):
                if kt == 0:
                    return x0h[c][:]
                return xt[kt][:, c * NB : (c + 1) * NB]

            # PE warm-up: ramps the clock while the head DMAs fly.
            wu = pp.tile([P, NB], F32, tag="ps", name="warm")
            for _ in range(4):
                nc.tensor.matmul(
                    wu[:], warm[:, 0:P], warm[:, 0:NB], start=True, stop=True
                )

            # Non-critical small constants on gpsimd (tiny transfers).
            b2t = cpool.tile([P, JT], F32, tag="b2")
            nc.gpsimd.dma_start(b2t[:], B2)
            b3t = cpool.tile([P, JT], F32, tag="b3")
            nc.gpsimd.dma_start(b3t[:], B3)
            # -w0/128 replicated; contracted against a ones column block so
            # the diag PSUM group finishes as (0.5*diag - w0).
            w0c = cpool.tile([P, HEADS], BF16, tag="w0c")
            nc.gpsimd.dma_start(w0c[:], W0C)
            onest = cpool.tile([P, P], BF16, tag="ones")
            nc.vector.memset(onest[:], 1.0)

            # Heavy weights in strict priority order on sync's ring.
            def load_w(dram, ktiles, name):
                ts = []
                for k in range(ktiles):
                    w_k = wpool.tile([P, HID], BF16, tag="w", name=f"{name}_{k}")
                    nc.sync.dma_start(w_k[:], dram[k * P : (k + 1) * P, :])
                    ts.append(w_k)
                return ts

            w2t = load_w(W2, KT, "w2")
            w3t = load_w(W3, KT, "w3")
            vtp = []
            for kp in range(KT // 2):
                v_k = wpool.tile([P, 2 * HR], BF16, tag="w", name=f"vt{kp}")
                nc.sync.dma_start(
                    v_k[:], VTP[:, kp * 2 * HR : (kp + 1) * 2 * HR]
                )
                vtp.append(v_k)

            def vtt(kt):                       # [128, HR] view of V^T k-tile
                return vtp[kt // 2][:, (kt % 2) * HR : (kt % 2 + 1) * HR]

            fwsq = wpool.tile([P, 2 * KT * HEADS], BF16, tag="w", name="fwsq")
            nc.sync.dma_start(fwsq[:], FWSQ)
            fwt = fwsq[:, 0 : KT * HEADS]
            sqt = fwsq[:, KT * HEADS : 2 * KT * HEADS]

            # --- embedder layers ------------------------------------------
            def layer(rhs_fn, lhsT_fn, bias_t, ktiles, name, rot):
                h_out = []
                for jt in range(JT):
                    ps = []
                    for c in range(NBC):
                        ps_c = pp.tile([P, NB], F32, tag="ps", name=f"{name}ps{jt}_{c}")
                        ps.append(ps_c)
                    # Rotate the accumulation start by jt (mod rot) so every
                    # jt can begin with an already-arrived k-tile while later
                    # tiles stream in.
                    kts = [(kt + jt) % rot for kt in range(rot)] + list(
                        range(rot, ktiles)
                    )
                    for i, kt in enumerate(kts):
                        lhsT = lhsT_fn(kt, jt)
                        for c in range(NBC):
                            nc.tensor.matmul(
                                ps[c][:],
                                lhsT,
                                rhs_fn(kt, c),
                                start=(i == 0),
                                stop=(i == ktiles - 1),
                            )
                    ht = hpool.tile([P, BC], BF16, tag="h", name=f"{name}h{jt}")
                    for c in range(NBC):
                        nc.scalar.activation(
                            ht[:, c * NB : (c + 1) * NB],
                            ps[c][:],
                            AF.Tanh,
                            bias=bias_t[:, jt : jt + 1],
                        )
                    h_out.append(ht)
                return h_out

            def w_lhsT(tiles):
                return lambda kt, jt: tiles[kt][:, jt * P : (jt + 1) * P]

            def h_rhs(tiles):
                return lambda kt, c: tiles[kt][:, c * NB : (c + 1) * NB]

            h1 = layer(l1_rhs, l1_lhsT, b1t, KT1, "l1", rot=1)
            h2 = layer(h_rhs(h1), w_lhsT(w2t), b2t, KT, "l2", rot=KT)
            h3 = layer(h_rhs(h2), w_lhsT(w3t), b3t, KT, "l3", rot=KT)

            # --- h3 squared (stationary operand for the diag matmuls) -----
            h3sq = []
            for k in range(KT):
                sq_k = hpool.tile([P, BC], BF16, tag="h", name=f"h3sq{k}")
                nc.vector.tensor_mul(sq_k[:], h3[k][:], h3[k][:])
                h3sq.append(sq_k)

            # --- FM stage: per 128-row batch tile -------------------------
            def fm_phase_a(bt):
                """vx = h V^T (1024 cols) and lin = h fm_w^T (64 cols)."""
                vx0 = pp.tile([P, NB], F32, tag="ps", name=f"vx0_{bt}")
                vx1 = pp.tile([P, NB], F32, tag="ps", name=f"vx1_{bt}")
                lw = pp.tile([P, NB], F32, tag="ps", name=f"lw_{bt}")
                bsl = slice(bt * P, (bt + 1) * P)
                for kt in range(KT):
                    lhsT = h3[kt][:, bsl]
                    vt_k = vtt(kt)
                    nc.tensor.matmul(
                        vx0[:], lhsT, vt_k[:, 0:NB],
                        start=(kt == 0), stop=(kt == KT - 1),
                    )
                    nc.tensor.matmul(
                        vx1[:], lhsT, vt_k[:, NB:HR],
                        start=(kt == 0), stop=(kt == KT - 1),
                    )
                    nc.tensor.matmul(
                        lw[:, 0:HEADS], lhsT,
                        fwt[:, kt * HEADS : (kt + 1) * HEADS],
                        start=(kt == 0), stop=(kt == KT - 1),
                    )
                return vx0, vx1, lw

            def fm_phase_b(bt):
                """diag = (h*h) . (0.5 * sum_r V^2), already scaled by 0.5."""
                dg = pp.tile([P, NB], F32, tag="ps", name=f"dg_{bt}")
                bsl = slice(bt * P, (bt + 1) * P)
                for kt in range(KT):
                    nc.tensor.matmul(
                        dg[:, 0:HEADS],
                        h3sq[kt][:, bsl],
                        sqt[:, kt * HEADS : (kt + 1) * HEADS],
                        start=(kt == 0), stop=False,
                    )
                nc.tensor.matmul(
                    dg[:, 0:HEADS], onest[:], w0c[:], start=False, stop=True,
                )
                return dg

            def fm_square_reduce(bt, vx0, vx1):
                """Emitted right after phase A: overlaps later bt's matmuls.
                Each 512-wide half squares then reduces independently so the
                two chains pipeline across ACT and DVE."""
                vx2 = epool.tile([P, HR], BF16, tag="e", name=f"vx2_{bt}")
                sumv = spool.tile([P, HEADS], F32, tag="s", name=f"sumv_{bt}")
                for c, vxh in ((0, vx0), (1, vx1)):
                    nc.scalar.activation(vx2[:, c * NB : (c + 1) * NB], vxh[:], AF.Square)
                    nc.vector.reduce_sum(
                        sumv[:, c * (HEADS // 2) : (c + 1) * (HEADS // 2)],
                        vx2[:, c * NB : (c + 1) * NB].rearrange(
                            "p (h r) -> p h r", r=RANK
                        ),
                        axis=mybir.AxisListType.X,
                    )
                return sumv

            def fm_combine(bt, sumv, lw, dg):
                # q = 0.5*sumv - diag_half
                q = spool.tile([P, HEADS], F32, tag="s", name=f"q_{bt}")
                nc.vector.scalar_tensor_tensor(
                    q[:], sumv[:], 0.5, dg[:, 0:HEADS],
                    op0=ALU.mult, op1=ALU.subtract,
                )
                ot = opool.tile([P, HEADS], F32, tag="o", name=f"ot_{bt}")
                nc.vector.tensor_add(ot[:], q[:], lw[:, 0:HEADS])
                nc.sync.dma_start(OUT[bt * P : (bt + 1) * P, :], ot[:])

            # Stagger: A(0), A(1), B(0), C(0), A(2), B(1), C(1), ...
            pend = []  # (bt, sumv, lw)
            for bt in range(BT):
                vx0, vx1, lw = fm_phase_a(bt)
                sumv = fm_square_reduce(bt, vx0, vx1)
                pend.append((bt, sumv, lw))
                if len(pend) == 2:
                    obt, osumv, olw = pend.pop(0)
                    dg = fm_phase_b(obt)
                    fm_combine(obt, osumv, olw, dg)
            while pend:
                obt, osumv, olw = pend.pop(0)
                dg = fm_phase_b(obt)
                fm_combine(obt, osumv, olw, dg)

    nc.compile()
    return nc


def _get_nc():
    if "nc" not in _CACHE:
        _CACHE["nc"] = _build_module()
    return _CACHE["nc"]


def _prep_host(x, W1, b1, W2, b2, W3, b3, fm_w0, fm_w, fm_V):
    """Host-side layout prep: bf16 casts, transposes, per-head V reductions."""
    bf = ml_dtypes.bfloat16
    f32 = np.float32

    # V^T: [2048, heads*rank], col hr = h*RANK + r; then k-pair packed
    VT = fm_V.reshape(HEADS * RANK, HID).T.astype(bf)          # [2048, 1024]
    VTP = np.empty((P, (KT // 2) * 2 * HR), dtype=bf)
    for kp in range(KT // 2):
        for i in range(2):
            VTP[:, kp * 2 * HR + i * HR : kp * 2 * HR + (i + 1) * HR] = VT[
                (2 * kp + i) * P : (2 * kp + i + 1) * P, :
            ]

    # fm_w^T packed as [128, kt*64]: FW[p, kt*64+h] = fm_w[h, kt*128+p]
    FW = (
        fm_w.T.reshape(KT, P, HEADS).transpose(1, 0, 2).reshape(P, KT * HEADS)
        .astype(bf)
    )
    # 0.5 * sum_r V^2, same packing
    SQ = (
        (0.5 * (fm_V.astype(np.float64) ** 2).sum(axis=1))
        .T.reshape(KT, P, HEADS).transpose(1, 0, 2).reshape(P, KT * HEADS)
        .astype(bf)
    )
    common = {
        "W1": np.ascontiguousarray(W1.astype(bf)),
        "W2": np.ascontiguousarray(W2.astype(bf)),
        "W3": np.ascontiguousarray(W3.astype(bf)),
        "B1": np.ascontiguousarray(b1.astype(f32).reshape(JT, P).T),
        "B2": np.ascontiguousarray(b2.astype(f32).reshape(JT, P).T),
        "B3": np.ascontiguousarray(b3.astype(f32).reshape(JT, P).T),
        "VTP": np.ascontiguousarray(VTP),
        "FWSQ": np.ascontiguousarray(np.concatenate([FW, SQ], axis=1)),
        "W0C": np.ascontiguousarray(
            np.tile((-fm_w0.astype(np.float64) / P)[None, :], (P, 1)).astype(bf)
        ),
    }

    in_maps = []
    xb = x.astype(bf)
    for c in range(NCORES):
        m = dict(common)
        m["xT"] = np.ascontiguousarray(xb[c * BC : (c + 1) * BC, :].T)
        in_maps.append(m)
    return in_maps


def kernel(x, W1, b1, W2, b2, W3, b3, fm_w0, fm_w, fm_V):
    # Host prep is plain numpy; coerce eagerly in case inputs are jax arrays.
    x, W1, b1, W2, b2, W3, b3, fm_w0, fm_w, fm_V = (
        np.asarray(a) for a in (x, W1, b1, W2, b2, W3, b3, fm_w0, fm_w, fm_V)
    )
    nc = _get_nc()
    in_maps = _prep_host(x, W1, b1, W2, b2, W3, b3, fm_w0, fm_w, fm_V)
    import os
    trace = bool(int(os.environ.get("KERNEL_TRACE", "0")))
    last_err = None
    for _attempt in range(3):
        try:
            res = bass_utils.run_bass_kernel_spmd(
                nc, in_maps, core_ids=list(range(NCORES)), trace=trace,
            )
            outs = [np.asarray(res.results[c]["out"]) for c in range(NCORES)]
            break
        except Exception as e:  # transient device faults (NRT unrecoverable)
            last_err = e
    else:
        raise last_err
    _CACHE["last_results"] = res
    full = np.concatenate(outs, axis=0)          # [B, HEADS]
    return np.ascontiguousarray(full.T).astype(np.float32)  # [HEADS, B]
